# revision 2
# baseline (speedup 1.0000x reference)
"""Correlation-network kernel for TRN2, batch-sharded over 8 NeuronCores.

Per core (one batch element b):
  A = feature_A[b] as [HW=2304, C=256], B = feature_B[b] likewise.
  out[m, n] = corr_raw[m, n] * s[n]
  where corr_raw = A @ B^T  and  s[n] = 1/sqrt(sum_m corr_raw[m, n]^2).
  The 1/C of the reference cancels between corr and penalty.

Column norms via the Gram chain: sum_m corr_raw[m,n]^2 = b_n^T (A^T A) b_n,
so G = A^T A ([256,256]) gives pen2 = colsum(B^T o (G B^T)) without a second
pass over the [2304,2304] output. The scale s is folded into B^T's columns so
the main GEMM directly emits scaled output.

v3 vs v2:
  - The whole norm chain runs in fp8e4m3 with MatmulPerfMode.DoubleRow
    (2 k-tiles per instruction, 0.5 cyc/row): G, Q = G B^T and the colsum
    drop from ~9.5 us to ~2.4 us of PE time. G is scaled by 1/256 in the
    psum->sbuf copy so Q/r stay inside fp8 range; the rsqrt activation's
    input scale of 256 compensates exactly. The norm chain only shapes the
    per-column scale s (~0.3% error budget), the main GEMM stays bf16.
  - a (natural-layout A) now arrives as fp8 (only feeds G), and bt8 is cast
    from bt on ACT instead of being DMA'd: input bytes drop 3.54->2.95 MB.
  - The s broadcast runs on the otherwise-idle Pool engine
    (gpsimd.partition_broadcast) instead of a PE matmul, and the bts muls
    are all-bf16 (DVE 2x mode).
  - Panel drains split DVE/ACT ~4:7 to balance the DVE-side muls.
"""
import numpy as np

B, H, W, C = 8, 48, 48, 256
HW = H * W            # 2304
MT = HW // 128        # 18 m-tiles
T2 = MT // 2          # 9 paired m-tiles for DoubleRow G
CHUNKS = [(0, 512), (512, 512), (1024, 512), (1536, 512), (2048, 256)]

_CACHE = {}


def _build(reps=1):
    import concourse.bacc as bacc
    import concourse.mybir as mybir
    import concourse.tile as tile

    dt = mybir.dt
    f32 = dt.float32
    bf16 = dt.bfloat16
    fp8 = dt.float8e4
    DR = mybir.MatmulPerfMode.DoubleRow

    nc = bacc.Bacc(None, target_bir_lowering=False, debug=False)
    # Partition-major swizzled inputs (see marshal_inputs):
    #   a8 [p, (t2*2+j)*C+c] = A[(t2*2+j)*128+p, c]   (fp8, DoubleRow pairs)
    #   at [p, h*HW+n]       = A[n, h*128+p]          (A^T, GEMM lhsT source)
    #   bt [p, h*HW+n]       = B[n, h*128+p]          (B^T, chain + GEMM rhs)
    a8_dram = nc.dram_tensor("a8", [128, MT * C], fp8, kind="ExternalInput")
    at_dram = nc.dram_tensor("at", [128, 2 * HW], bf16, kind="ExternalInput")
    bt_dram = nc.dram_tensor("bt", [128, 2 * HW], bf16, kind="ExternalInput")
    o_dram = nc.dram_tensor("out", [HW, HW], bf16, kind="ExternalOutput")
    o_r = o_dram[:, :].rearrange("(t p) n -> p t n", p=128)

    with tile.TileContext(nc) as tc, nc.allow_low_precision(
            reason="bf16/fp8 pipeline is intentional; l2 tolerance is 2e-2"):
        consts = tc.alloc_tile_pool(name="consts", bufs=1)
        ones_f = consts.tile([128, 2, 1], f32)
        nc.vector.memset(ones_f, 1.0)
        ones8 = consts.tile([128, 2, 1], fp8)
        nc.vector.tensor_copy(ones8, ones_f)

        inp = tc.alloc_tile_pool(name="inp", bufs=2)
        sca = tc.alloc_tile_pool(name="sca", bufs=2)
        scr = tc.alloc_tile_pool(name="scr", bufs=3)
        panels = tc.alloc_tile_pool(name="panels", bufs=10)
        ps_gq = tc.alloc_tile_pool(name="ps_gq", bufs=2, space="PSUM")
        ps_pb = tc.alloc_tile_pool(name="ps_pb", bufs=1, space="PSUM")
        ps_mm = tc.alloc_tile_pool(name="ps_mm", bufs=5, space="PSUM")

        NCH = len(CHUNKS)

        def make_tiles():
            a8 = inp.tile([128, T2, 2, C], fp8, tag="a8", name="a8")
            at = inp.tile([128, 2 * HW], bf16, tag="at", name="at")
            bt = inp.tile([128, 2 * HW], bf16, tag="bt", name="bt")
            bt8 = inp.tile([128, 2, HW], fp8, tag="bt8", name="bt8")
            g8 = sca.tile([128, 2, C], fp8, tag="g", name="g8")
            s_bf = sca.tile([1, HW], bf16, tag="s", name="s")
            sbc = sca.tile([128, HW], bf16, tag="sbc", name="sbc")
            bts = sca.tile([128, 2 * HW], bf16, tag="bts", name="bts")
            return dict(a8=a8, at=at, bt=bt, bt8=bt8, g8=g8, s_bf=s_bf,
                        sbc=sbc, bts=bts)

        def emit_dmas(tl):
            # a8 first: G gates the chain's critical path. Each partition
            # line >= 2KB keeps DMA at line rate (a8 4608B, halves 4608B).
            nc.sync.dma_start(out=tl["a8"], in_=a8_dram[:, :].rearrange(
                "p (t j c) -> p t j c", t=T2, j=2))
            for h in (0, 1):
                nc.sync.dma_start(out=tl["bt"][:, h * HW:(h + 1) * HW],
                                  in_=bt_dram[:, h * HW:(h + 1) * HW])
            nc.sync.dma_start(out=tl["at"], in_=at_dram[:, :])

        def emit_g(tl):
            # G = A^T A ([256,256], fp8 DoubleRow: 2 m-tiles per matmul).
            # Both c-halves of G computed directly (no symmetric transpose:
            # Q's lhsT wants G[k, m] = G[m, k] anyway).
            # g8[p, j, c'] = G[j*128+p, c'] / 256, scaled into fp8 range.
            a8, g8 = tl["a8"], tl["g8"]
            pg = ps_gq.tile([128, 512], f32, tag="pgq", name="pg")
            for j2 in (0, 1):
                for t in range(T2):
                    nc.tensor.matmul(
                        pg[:, j2 * C:(j2 + 1) * C],
                        a8[:, t, :, j2 * 128:(j2 + 1) * 128],
                        a8[:, t, :, :],
                        start=(t == 0), stop=(t == T2 - 1),
                        perf_mode=DR)
            nc.scalar.activation(
                g8[:, :, :].rearrange("p j c -> p (j c)"), pg[:, :],
                mybir.ActivationFunctionType.Copy, scale=1.0 / 256)
            # bt8: fp8 cast of B^T for the DoubleRow chain (saves the DMA)
            for h in (0, 1):
                nc.scalar.copy(tl["bt8"][:, h, :],
                               tl["bt"][:, h * HW:(h + 1) * HW])

        def chunk_pipe(tl, ci):
            # pq_j = (G/256) B^T (one DoubleRow matmul per c-half);
            # r[:,j,:] = B^T o pq_j (fp8); pen2 = DoubleRow-colsum(r);
            # s = rsqrt(256 * pen2/256); sbc = broadcast s (Pool);
            # bts = B^T * s (all-bf16, DVE 2x)
            bt, bt8, g8 = tl["bt"], tl["bt8"], tl["g8"]
            s_bf, sbc, bts = tl["s_bf"], tl["sbc"], tl["bts"]
            n0, cw = CHUNKS[ci]
            r = scr.tile([128, 2, cw], fp8, tag=f"r{ci % 3}", name="r")
            for j in (0, 1):
                pq = ps_gq.tile([128, 512], f32, tag="pgq", name="pq")
                nc.tensor.matmul(
                    pq[:, :cw],
                    g8[:, :, j * 128:(j + 1) * 128],
                    bt8[:, :, n0:n0 + cw],
                    start=True, stop=True, perf_mode=DR)
                nc.vector.tensor_mul(
                    r[:, j, :], bt[:, j * HW + n0:j * HW + n0 + cw],
                    pq[:, :cw])
            pp = ps_pb.tile([1, 512], f32, tag="ppb", name="pp")
            nc.tensor.matmul(pp[:, :cw], ones8, r[:, :, :],
                             start=True, stop=True, perf_mode=DR)
            # s = 1/sqrt(pen2): ACT rsqrt with input scale 256 undoing the
            # g8 scaling exactly (pen2 >= 0 so |x| = x).
            nc.scalar.activation(
                s_bf[:, n0:n0 + cw], pp[:, :cw],
                mybir.ActivationFunctionType.Abs_reciprocal_sqrt, scale=256.0)
            nc.gpsimd.partition_broadcast(sbc[:, n0:n0 + cw],
                                          s_bf[0:1, n0:n0 + cw])
            for h in (0, 1):
                nc.vector.tensor_mul(
                    bts[:, h * HW + n0:h * HW + n0 + cw],
                    bt[:, h * HW + n0:h * HW + n0 + cw],
                    sbc[:, n0:n0 + cw])

        # Software pipeline across reps: rep r+1's input DMAs are issued at
        # wavefront step 4 of rep r (the in-order HWDGE queue reaches them
        # mid-body instead of after all of rep r's panel DMAs), and rep r+1's
        # G matmuls are emitted at step 12 (the PE FIFO reaches them when
        # a8 has long landed, so G runs gap-free inside rep r's stream).
        tiles = make_tiles()
        emit_dmas(tiles)
        emit_g(tiles)
        pipes_pre = False
        for _rep in range(reps):
            tl = tiles
            nxt = None

            # main GEMM on a diagonal wavefront: step k emits (mt, ci) with
            # mt = k - ci, so program order (= PE FIFO order) only ever needs
            # chunk ci about k*2us after GEMM start; chunk ci+1's scale pipe
            # is emitted just ahead of the step that first consumes it (rep 0
            # only -- later reps' pipes were pre-run in the previous body, so
            # their wavefronts start with every bts chunk ready).
            at, bts = tl["at"], tl["bts"]
            if not pipes_pre:
                chunk_pipe(tl, 0)
            panel_by_mt = {}
            for k in range(MT + NCH - 1):
                if not pipes_pre and k + 1 < NCH:
                    chunk_pipe(tl, k + 1)
                if k == 4 and _rep + 1 < reps:
                    nxt = make_tiles()
                    emit_dmas(nxt)
                if k == 12 and nxt is not None:
                    emit_g(nxt)
                if nxt is not None and 14 <= k < 14 + NCH:
                    chunk_pipe(nxt, k - 14)
                for ci in range(NCH):
                    mt = k - ci
                    if not (0 <= mt < MT):
                        continue
                    n0, cw = CHUNKS[ci]
                    if ci == 0:
                        panel_by_mt[mt] = panels.tile([128, HW], bf16,
                                                      tag="panel",
                                                      name="panel")
                    panel = panel_by_mt[mt]
                    # 5 pm slots = one per chunk column: each column
                    # double-buffers against its own previous m-tile and the
                    # wavefront never touches the pipe pools
                    pm = ps_mm.tile([128, 512], f32, tag="pm", name="pm")
                    for h in (0, 1):
                        nc.tensor.matmul(
                            pm[:, :cw],
                            at[:, h * HW + mt * 128:h * HW + (mt + 1) * 128],
                            bts[:, h * HW + n0:h * HW + n0 + cw],
                            start=(h == 0), stop=(h == 1))
                    # ~4/11 of panel copies on DVE, rest on ACT: DVE also
                    # carries the chunk-pipeline muls.
                    cp = (nc.vector.tensor_copy if (mt * 5 + ci) % 11 < 4
                          else nc.scalar.copy)
                    cp(panel[:, n0:n0 + cw], pm[:, :cw])
                    # split the panel store: the first piece fires three
                    # steps early, smoothing the write stream; both pieces
                    # keep partition lines >= 2KB for DMA line rate
                    if ci == 1:
                        nc.sync.dma_start(out=o_r[:, mt, :1024],
                                          in_=panel[:, :1024])
                    elif ci == NCH - 1:
                        nc.sync.dma_start(out=o_r[:, mt, 1024:],
                                          in_=panel[:, 1024:])
            if nxt is not None:
                tiles = nxt
                pipes_pre = True

        for pool in (ps_mm, ps_pb, ps_gq,
                     panels, scr, sca, inp, consts):
            pool.release()
    nc.finalize()
    return nc


def _get_nc(reps=1):
    key = ("nc", reps)
    if key not in _CACHE:
        _CACHE[key] = _build(reps)
    return _CACHE[key]


def marshal_inputs(feature_A, feature_B):
    """Full f32 inputs -> per-core partition-major bf16/fp8 arrays."""
    import ml_dtypes
    bf = ml_dtypes.bfloat16
    f8 = ml_dtypes.float8_e4m3
    fa = np.asarray(feature_A, dtype=np.float32).reshape(B, HW, C)
    fb = np.asarray(feature_B, dtype=np.float32).reshape(B, HW, C)
    # a8[b, p, (t2*2+j)*C+c] = A[b, (t2*2+j)*128+p, c]
    a8 = np.ascontiguousarray(
        fa.astype(f8).reshape(B, MT, 128, C).transpose(0, 2, 1, 3)
    ).reshape(B, 128, MT * C)
    # at[b, p, h*HW+n] = A[b, n, h*128+p]
    at_sw = np.ascontiguousarray(
        fa.astype(bf).reshape(B, HW, 2, 128).transpose(0, 3, 2, 1)
    ).reshape(B, 128, 2 * HW)
    bt_sw = np.ascontiguousarray(
        fb.astype(bf).reshape(B, HW, 2, 128).transpose(0, 3, 2, 1)
    ).reshape(B, 128, 2 * HW)
    return a8, at_sw, bt_sw


def run(feature_A, feature_B, trace=False):
    from concourse.bass_utils import run_bass_kernel_spmd

    nc = _get_nc()
    a8, at_sw, bt_sw = marshal_inputs(feature_A, feature_B)
    in_maps = [{"a8": a8[i], "at": at_sw[i], "bt": bt_sw[i]}
               for i in range(B)]
    res = run_bass_kernel_spmd(nc, in_maps, list(range(B)), trace=trace)
    out = np.stack([res.results[i]["out"].astype(np.float32)
                    for i in range(B)])
    return out.reshape(B, H, W, H, W), res


def kernel(feature_A, feature_B):
    out, _ = run(feature_A, feature_B)
    return out
